# revision 10
# baseline (speedup 1.0000x reference)
import sys
sys.path.insert(0, "/opt/trn_rl_repo")
import numpy as np
import ml_dtypes
import concourse.bacc as bacc
import concourse.mybir as mybir
from concourse.tile import TileContext
from concourse.masks import make_identity

N_CORES = 8
B, H, W, C = 16, 256, 256, 64
BPC = B // N_CORES  # batches per core
F32 = mybir.dt.float32
BF16 = mybir.dt.bfloat16
BF_NP = ml_dtypes.bfloat16

_CACHE = {}


def _constants():
    t = np.arange(128)
    h = np.arange(256)
    out = {}
    for hf in range(2):
        ang = 2 * np.pi * (((t[None, :] + 128 * hf) * h[:, None]) % 256) / 256
        out[f"ch{hf}"] = np.cos(ang).astype(np.float32).astype(BF_NP)  # [h, t]
        out[f"sh{hf}"] = (-np.sin(ang)).astype(np.float32).astype(BF_NP)
    qm = np.fft.irfft(1j * np.fft.rfft(np.eye(256), axis=1), n=256, axis=1)
    out["qm"] = qm.astype(np.float32).astype(BF_NP)  # [w_in, w_out]
    wv = np.arange(256)
    qv = np.arange(32)
    phi = 2 * np.pi * np.outer(wv, qv) / 256  # [w, q]
    out["cw"] = np.cos(phi).astype(np.float32).astype(BF_NP)
    out["sw"] = np.sin(phi).astype(np.float32).astype(BF_NP)
    out["swn"] = (-np.sin(phi)).astype(np.float32).astype(BF_NP)
    return out


def _build():
    nc = bacc.Bacc()
    xs = nc.dram_tensor("xs", [BPC, H, W, C], BF16, kind="ExternalInput")
    ch0 = nc.dram_tensor("ch0", [256, 128], BF16, kind="ExternalInput")
    ch1 = nc.dram_tensor("ch1", [256, 128], BF16, kind="ExternalInput")
    sh0 = nc.dram_tensor("sh0", [256, 128], BF16, kind="ExternalInput")
    sh1 = nc.dram_tensor("sh1", [256, 128], BF16, kind="ExternalInput")
    qm = nc.dram_tensor("qm", [256, 256], BF16, kind="ExternalInput")
    cw = nc.dram_tensor("cw", [256, 32], BF16, kind="ExternalInput")
    sw = nc.dram_tensor("sw", [256, 32], BF16, kind="ExternalInput")
    swn = nc.dram_tensor("swn", [256, 32], BF16, kind="ExternalInput")
    out = nc.dram_tensor("out", [BPC, H, W, C], BF16, kind="ExternalOutput")
    # zout[b, hf, q, 0:2048] = Zre[q, c*32+t], [..., 2048:4096] = Zim
    zout = nc.dram_tensor("zout", [BPC, 2, 32, 4096], BF16, kind="ExternalOutput")
    chs = {0: ch0, 1: ch1}
    shs = {0: sh0, 1: sh1}

    with TileContext(nc) as tc:
        with tc.tile_pool(name="const", bufs=1) as cpool, \
             tc.tile_pool(name="big", bufs=1) as bigpool, \
             tc.tile_pool(name="xin", bufs=4) as xpool, \
             tc.tile_pool(name="work", bufs=1) as wpool, \
             tc.tile_pool(name="ob", bufs=2) as opool, \
             tc.tile_pool(name="ps", bufs=2, space="PSUM") as pspool, \
             tc.tile_pool(name="psv", bufs=2, space="PSUM") as psvpool, \
             tc.tile_pool(name="psz", bufs=2, space="PSUM") as pszpool:

            ident = cpool.tile([128, 128], BF16, tag="ident")
            make_identity(nc, ident[:])
            cons = {}
            for hf in range(2):
                for nm, src in (("ch", chs[hf]), ("sh", shs[hf])):
                    tl = cpool.tile([128, 256], BF16, tag=f"{nm}{hf}")
                    # [K=h(2x128 chunks), M=128t] stored as [128, 2*128]
                    nc.sync.dma_start(
                        out=tl[:].rearrange("p (k m) -> p k m", k=2),
                        in_=src[:].rearrange("(k p) m -> p k m", k=2))
                    cons[f"{nm}{hf}"] = tl
            qmt = cpool.tile([128, 512], BF16, tag="qm")
            nc.sync.dma_start(
                out=qmt[:].rearrange("p (k m) -> p k m", k=2),
                in_=qm[:].rearrange("(k p) m -> p k m", k=2))
            # rfft-mode tables [K=w (2x128 chunks), M=32q] as [128, 2*32]
            zcons = {}
            for nm, src in (("cw", cw), ("sw", sw), ("swn", swn)):
                tl = cpool.tile([128, 64], BF16, tag=nm)
                nc.sync.dma_start(
                    out=tl[:].rearrange("p (k q) -> p k q", k=2),
                    in_=src[:].rearrange("(k p) q -> p k q", k=2))
                zcons[nm] = tl

            for b in range(BPC):
                for hf in range(2):
                    # ---------------- phase B: contract h ----------------
                    yre = bigpool.tile([128, 16384], F32, tag="yre")
                    yim = bigpool.tile([128, 16384], BF16, tag="yim")
                    for wb in range(64):
                        xt = xpool.tile([128, 512], BF16, tag="xt")
                        # [h=128p x2 chunks, (4w,64c)=256]
                        nc.sync.dma_start(
                            out=xt[:].rearrange("p (k w c) -> p k w c", k=2, w=4),
                            in_=xs[b, :, 4 * wb:4 * wb + 4, :]
                            .rearrange("(k p) w c -> p k w c", k=2))
                        pre = pspool.tile([128, 256], F32, tag="pre")
                        pim = pspool.tile([128, 256], F32, tag="pim")
                        ct, st = cons[f"ch{hf}"], cons[f"sh{hf}"]
                        nc.tensor.matmul(pre[:], ct[:, 0:128], xt[:, 0:256],
                                         start=True, stop=False)
                        nc.tensor.matmul(pre[:], ct[:, 128:256], xt[:, 256:512],
                                         start=False, stop=True)
                        nc.tensor.matmul(pim[:], st[:, 0:128], xt[:, 0:256],
                                         start=True, stop=False)
                        nc.tensor.matmul(pim[:], st[:, 128:256], xt[:, 256:512],
                                         start=False, stop=True)
                        if wb % 2 == 0:
                            nc.vector.tensor_copy(
                                yre[:, 256 * wb:256 * wb + 256], pre[:])
                            nc.scalar.copy(
                                yim[:, 256 * wb:256 * wb + 256], pim[:])
                        else:
                            nc.scalar.copy(
                                yre[:, 256 * wb:256 * wb + 256], pre[:])
                            nc.vector.tensor_copy(
                                yim[:, 256 * wb:256 * wb + 256], pim[:])

                    # ------- Q path + Z modes per c-group of 16 -------
                    trow = 0 if hf == 0 else 96
                    zsb = wpool.tile([32, 4096], BF16, tag="zsb")
                    for cg in range(4):
                        yg = wpool.tile([128, 4096], BF16, tag="yg")
                        # regroup: yg[t, ci*256 + w] = yim[t, w*64 + (16cg+ci)]
                        nc.vector.tensor_copy(
                            yg[:].rearrange("p (c w) -> p c w", c=16),
                            yim[:].rearrange("p (w c) -> p c w", c=64)
                            [:, 16 * cg:16 * cg + 16, :])
                        ygr = wpool.tile([128, 4096], BF16, tag="ygr")
                        nc.scalar.copy(
                            ygr[:].rearrange("p (c w) -> p c w", c=16),
                            yre[:].rearrange("p (w c) -> p c w", c=64)
                            [:, 16 * cg:16 * cg + 16, :])
                        ytr = wpool.tile([128, 2048], BF16, tag="ytr0")
                        ytr1 = wpool.tile([128, 2048], BF16, tag="ytr1")
                        # yre^T slices for Z: [w-chunk, ci*32+t(32)]
                        ztr = wpool.tile([128, 512], BF16, tag="ztr0")
                        ztr1 = wpool.tile([128, 512], BF16, tag="ztr1")
                        for ci in range(16):
                            for k in range(2):
                                ptr = psvpool.tile([128, 128], BF16, tag="ptr")
                                nc.tensor.transpose(
                                    ptr[:],
                                    yg[:, 256 * ci + 128 * k:256 * ci + 128 * k + 128],
                                    ident[:])
                                dst = ytr if k == 0 else ytr1
                                nc.vector.tensor_copy(
                                    dst[:, 128 * ci:128 * ci + 128], ptr[:])
                                ptz = psvpool.tile([128, 128], BF16, tag="ptr")
                                nc.tensor.transpose(
                                    ptz[:],
                                    ygr[:, 256 * ci + 128 * k:256 * ci + 128 * k + 128],
                                    ident[:])
                                dstz = ztr if k == 0 else ztr1
                                nc.scalar.copy(
                                    dstz[:, 32 * ci:32 * ci + 32],
                                    ptz[:, trow:trow + 32])
                        # Z matmuls for this c-group -> zsb cols [512cg:+512]
                        # rhs yim^T slices: strided view of ytr [w, (ci,t)]
                        yimt0 = ytr[:].rearrange("p (c t) -> p c t", c=16)[:, :, trow:trow + 32]
                        yimt1 = ytr1[:].rearrange("p (c t) -> p c t", c=16)[:, :, trow:trow + 32]
                        zr = pszpool.tile([32, 512], F32, tag="zz")
                        zi = pszpool.tile([32, 512], F32, tag="zz")
                        cwt, swt, swnt = zcons["cw"], zcons["sw"], zcons["swn"]
                        nc.tensor.matmul(zr[:], cwt[:, 0:32], ztr[:],
                                         start=True, stop=False)
                        nc.tensor.matmul(zr[:], cwt[:, 32:64], ztr1[:],
                                         start=False, stop=False)
                        nc.tensor.matmul(zr[:], swt[:, 0:32], yimt0,
                                         start=False, stop=False)
                        nc.tensor.matmul(zr[:], swt[:, 32:64], yimt1,
                                         start=False, stop=True)
                        nc.tensor.matmul(zi[:], cwt[:, 0:32], yimt0,
                                         start=True, stop=False)
                        nc.tensor.matmul(zi[:], cwt[:, 32:64], yimt1,
                                         start=False, stop=False)
                        nc.tensor.matmul(zi[:], swnt[:, 0:32], ztr[:],
                                         start=False, stop=False)
                        nc.tensor.matmul(zi[:], swnt[:, 32:64], ztr1[:],
                                         start=False, stop=True)
                        nc.vector.tensor_copy(
                            zsb[:, 512 * cg:512 * cg + 512], zr[:])
                        nc.scalar.copy(
                            zsb[:, 2048 + 512 * cg:2048 + 512 * cg + 512], zi[:])
                        # Q matmuls: add yim*Q into yre
                        for ci in range(16):
                            c = 16 * cg + ci
                            pv = pspool.tile([128, 256], F32, tag="pre")
                            nc.tensor.matmul(pv[:], ytr[:, 128 * ci:128 * ci + 128],
                                             qmt[:, 0:256], start=True, stop=False)
                            nc.tensor.matmul(pv[:], ytr1[:, 128 * ci:128 * ci + 128],
                                             qmt[:, 256:512], start=False, stop=True)
                            nc.vector.tensor_add(
                                yre[:].rearrange("p (w c) -> p c w", c=64)[:, c, :],
                                yre[:].rearrange("p (w c) -> p c w", c=64)[:, c, :],
                                pv[:])
                    nc.sync.dma_start(out=zout[b, hf], in_=zsb[:])
                    # out rows (bf16 cast, chunked staging)
                    for oc in range(4):
                        ob = opool.tile([128, 4096], BF16, tag="ob")
                        if oc % 2 == 0:
                            nc.vector.tensor_copy(ob[:], yre[:, 4096 * oc:4096 * oc + 4096])
                        else:
                            nc.scalar.copy(ob[:], yre[:, 4096 * oc:4096 * oc + 4096])
                        nc.sync.dma_start(
                            out=out[b, 128 * hf:128 * hf + 128,
                                    64 * oc:64 * oc + 64, :]
                            .rearrange("p w c -> p (w c)"),
                            in_=ob[:])
    nc.compile()
    return nc


def _get_runtime():
    if "rt" in _CACHE:
        return _CACHE["rt"]
    import jax
    from jax.sharding import Mesh, PartitionSpec, NamedSharding
    from jax.experimental.shard_map import shard_map
    from concourse.bass2jax import (
        _bass_exec_p, install_neuronx_cc_hook, partition_id_tensor)
    install_neuronx_cc_hook()
    nc = _build()
    partition_name = (nc.partition_id_tensor.name
                      if nc.partition_id_tensor is not None else None)
    in_names, out_names, out_avals = [], [], []
    for alloc in nc.m.functions[0].allocations:
        if not isinstance(alloc, mybir.MemoryLocationSet):
            continue
        assert alloc.memorylocations
        name = alloc.memorylocations[0].name
        if alloc.kind == "ExternalInput":
            if name != partition_name:
                in_names.append(name)
        elif alloc.kind == "ExternalOutput":
            shape = tuple(alloc.tensor_shape)
            dtype = mybir.dt.np(alloc.dtype)
            out_names.append(name)
            out_avals.append(jax.core.ShapedArray(shape, dtype))
    n_params = len(in_names)
    all_names = list(in_names) + list(out_names)
    if partition_name is not None:
        all_names.append(partition_name)

    def _body(*args):
        operands = list(args)
        if partition_name is not None:
            operands.append(partition_id_tensor())
        outs = _bass_exec_p.bind(
            *operands,
            out_avals=tuple(out_avals),
            in_names=tuple(all_names),
            out_names=tuple(out_names),
            lowering_input_output_aliases=(),
            sim_require_finite=True,
            sim_require_nnan=True,
            nc=nc,
        )
        return tuple(outs)

    devices = jax.devices()[:N_CORES]
    mesh = Mesh(np.asarray(devices), ("core",))
    fn = jax.jit(
        shard_map(_body, mesh=mesh,
                  in_specs=(PartitionSpec("core"),) * (n_params + len(out_names)),
                  out_specs=(PartitionSpec("core"),) * len(out_names),
                  check_rep=False),
        keep_unused=True)
    sh = NamedSharding(mesh, PartitionSpec("core"))
    cons = _constants()
    const_dev = {
        k: jax.device_put(np.concatenate([np.asarray(v)] * N_CORES, axis=0), sh)
        for k, v in cons.items()}
    zeros_dev = {}
    for name, aval in zip(out_names, out_avals):
        z = np.zeros((N_CORES * aval.shape[0],) + tuple(aval.shape[1:]),
                     aval.dtype)
        zeros_dev[name] = jax.device_put(z, sh)
    rt = (fn, in_names, out_names, sh, const_dev, zeros_dev)
    _CACHE["rt"] = rt
    return rt


def _get_wm(w1, w2):
    import zlib
    w1f = np.ascontiguousarray(w1, dtype=np.float32)
    w2f = np.ascontiguousarray(w2, dtype=np.float32)
    key = (zlib.adler32(memoryview(w1f).cast("B")),
           zlib.adler32(memoryview(w2f).cast("B")))
    hit = _CACHE.get("wm")
    if hit is not None and hit[0] == key:
        return hit[1]
    # einsum 'bctq,dctq->bdtq': weight axis0 = output channel d, axis1 = c
    w1c = (w1f[..., 0] + 1j * w1f[..., 1]).astype(np.complex64)
    w2c = (w2f[..., 0] + 1j * w2f[..., 1]).astype(np.complex64)
    Wm = np.stack([np.ascontiguousarray(w1c.transpose(2, 3, 1, 0)),
                   np.ascontiguousarray(w2c.transpose(2, 3, 1, 0))])  # blk,t,q,c,d
    _CACHE["wm"] = (key, Wm)
    return Wm


def _corr_rows(zraw, Wm):
    # zraw: [B, 2, 32, 4096] bf16; cols [0:2048]=Zre[q, c*32+t], [2048:]=Zim
    z = np.asarray(zraw).astype(np.float32)
    zre = z[..., :2048].reshape(B, 2, 32, 64, 32)  # b, blk, q, c, t
    zim = z[..., 2048:].reshape(B, 2, 32, 64, 32)
    Zc = zre + 1j * zim  # complex64
    A = np.ascontiguousarray(np.transpose(Zc, (1, 4, 2, 0, 3)))  # blk,t,q,b,c
    E = np.matmul(A, Wm)              # blk, t, q, b, d
    delta = (E - A).transpose(0, 1, 3, 2, 4)  # blk, t, b, q, d
    Xr = np.empty((2, 32, B, 64, 64), np.float32)
    Xr[..., 0::2, :] = delta.real
    Xr[..., 1::2, :] = delta.imag
    M = _CACHE.get("mext")
    if M is None:
        qv = np.arange(32)
        wv = np.arange(256)
        eps = (np.where(qv == 0, 1.0, 2.0) / 256.0).astype(np.float32)
        phi = 2 * np.pi * np.outer(wv, qv) / 256
        M = np.empty((256, 64), np.float32)
        M[:, 0::2] = np.cos(phi) * eps
        M[:, 1::2] = -np.sin(phi) * eps
        _CACHE["mext"] = M
    R = np.matmul(M, Xr)              # blk, t, b, w, d
    return R


def kernel(x, w1, w2):
    import jax
    import concurrent.futures as cf
    fn, in_names, out_names, sh, const_dev, zeros_dev = _get_runtime()
    # pipeline host bf16 cast with per-device async puts
    devs = jax.devices()[:N_CORES]
    x = np.asarray(x)
    shards = [jax.device_put(x[BPC * i:BPC * i + BPC].astype(BF_NP), devs[i])
              for i in range(N_CORES)]
    xdev = jax.make_array_from_single_device_arrays((B, H, W, C), sh, shards)
    args = [xdev if n == "xs" else const_dev[n] for n in in_names]
    args += [zeros_dev[n] for n in out_names]
    outs = fn(*args)
    # weight prep overlaps the x upload + exec (link busy, CPU idle here)
    Wm = _get_wm(w1, w2)
    omap = dict(zip(out_names, outs))
    out = np.empty((B, H, W, C), np.float32)
    zraw = np.empty((B, 2, 32, 4096), BF_NP)

    def _fetch_out(s):
        i = s.index[0].start
        out[i:i + BPC] = np.asarray(s.data)  # bf16 -> f32 cast in place

    def _fetch_z(s):
        i = s.index[0].start
        zraw[i:i + BPC] = np.asarray(s.data)

    # all shard fetches in parallel: overlaps per-shard RPC latency and
    # lets the corr einsum run while out shards stream in
    with cf.ThreadPoolExecutor(16) as ex:
        zf = [ex.submit(_fetch_z, s) for s in omap["zout"].addressable_shards]
        of = [ex.submit(_fetch_out, s) for s in omap["out"].addressable_shards]
        for f in zf:
            f.result()
        R = _corr_rows(zraw, Wm)
        for f in of:
            f.result()
    out[:, 0:32] += R[0].transpose(1, 0, 2, 3)
    out[:, 224:256] += R[1].transpose(1, 0, 2, 3)
    return out


# revision 11
# speedup vs baseline: 1.0935x; 1.0935x over previous
import sys
sys.path.insert(0, "/opt/trn_rl_repo")
import numpy as np
import ml_dtypes
import concourse.bacc as bacc
import concourse.mybir as mybir
from concourse.tile import TileContext
from concourse.masks import make_identity

N_CORES = 8
B, H, W, C = 16, 256, 256, 64
BPC = B // N_CORES  # batches per core
F32 = mybir.dt.float32
BF16 = mybir.dt.bfloat16
BF_NP = ml_dtypes.bfloat16

_CACHE = {}


def _constants():
    t = np.arange(128)
    h = np.arange(256)
    out = {}
    for hf in range(2):
        ang = 2 * np.pi * (((t[None, :] + 128 * hf) * h[:, None]) % 256) / 256
        out[f"ch{hf}"] = np.cos(ang).astype(np.float32).astype(BF_NP)  # [h, t]
        out[f"sh{hf}"] = (-np.sin(ang)).astype(np.float32).astype(BF_NP)
    qm = np.fft.irfft(1j * np.fft.rfft(np.eye(256), axis=1), n=256, axis=1)
    out["qm"] = qm.astype(np.float32).astype(BF_NP)  # [w_in, w_out]
    wv = np.arange(256)
    qv = np.arange(32)
    phi = 2 * np.pi * np.outer(wv, qv) / 256  # [w, q]
    out["cw"] = np.cos(phi).astype(np.float32).astype(BF_NP)
    out["sw"] = np.sin(phi).astype(np.float32).astype(BF_NP)
    out["swn"] = (-np.sin(phi)).astype(np.float32).astype(BF_NP)
    return out


def _build():
    nc = bacc.Bacc()
    xs = nc.dram_tensor("xs", [BPC, H, W, C], BF16, kind="ExternalInput")
    ch0 = nc.dram_tensor("ch0", [256, 128], BF16, kind="ExternalInput")
    ch1 = nc.dram_tensor("ch1", [256, 128], BF16, kind="ExternalInput")
    sh0 = nc.dram_tensor("sh0", [256, 128], BF16, kind="ExternalInput")
    sh1 = nc.dram_tensor("sh1", [256, 128], BF16, kind="ExternalInput")
    qm = nc.dram_tensor("qm", [256, 256], BF16, kind="ExternalInput")
    cw = nc.dram_tensor("cw", [256, 32], BF16, kind="ExternalInput")
    sw = nc.dram_tensor("sw", [256, 32], BF16, kind="ExternalInput")
    swn = nc.dram_tensor("swn", [256, 32], BF16, kind="ExternalInput")
    out = nc.dram_tensor("out", [BPC, H, W, C], BF16, kind="ExternalOutput")
    # zout[b, hf, q, 0:2048] = Zre[q, c*32+t], [..., 2048:4096] = Zim
    zout = nc.dram_tensor("zout", [BPC, 2, 32, 4096], BF16, kind="ExternalOutput")
    chs = {0: ch0, 1: ch1}
    shs = {0: sh0, 1: sh1}

    with TileContext(nc) as tc:
        with tc.tile_pool(name="const", bufs=1) as cpool, \
             tc.tile_pool(name="big", bufs=1) as bigpool, \
             tc.tile_pool(name="xin", bufs=4) as xpool, \
             tc.tile_pool(name="work", bufs=1) as wpool, \
             tc.tile_pool(name="ob", bufs=2) as opool, \
             tc.tile_pool(name="ps", bufs=2, space="PSUM") as pspool, \
             tc.tile_pool(name="psv", bufs=2, space="PSUM") as psvpool, \
             tc.tile_pool(name="psz", bufs=2, space="PSUM") as pszpool:

            ident = cpool.tile([128, 128], BF16, tag="ident")
            make_identity(nc, ident[:])
            cons = {}
            for hf in range(2):
                for nm, src in (("ch", chs[hf]), ("sh", shs[hf])):
                    tl = cpool.tile([128, 256], BF16, tag=f"{nm}{hf}")
                    # [K=h(2x128 chunks), M=128t] stored as [128, 2*128]
                    nc.sync.dma_start(
                        out=tl[:].rearrange("p (k m) -> p k m", k=2),
                        in_=src[:].rearrange("(k p) m -> p k m", k=2))
                    cons[f"{nm}{hf}"] = tl
            qmt = cpool.tile([128, 512], BF16, tag="qm")
            nc.sync.dma_start(
                out=qmt[:].rearrange("p (k m) -> p k m", k=2),
                in_=qm[:].rearrange("(k p) m -> p k m", k=2))
            # rfft-mode tables [K=w (2x128 chunks), M=32q] as [128, 2*32]
            zcons = {}
            for nm, src in (("cw", cw), ("sw", sw), ("swn", swn)):
                tl = cpool.tile([128, 64], BF16, tag=nm)
                nc.sync.dma_start(
                    out=tl[:].rearrange("p (k q) -> p k q", k=2),
                    in_=src[:].rearrange("(k p) q -> p k q", k=2))
                zcons[nm] = tl

            for b in range(BPC):
                for hf in range(2):
                    # ---------------- phase B: contract h ----------------
                    yre = bigpool.tile([128, 16384], F32, tag="yre")
                    yim = bigpool.tile([128, 16384], BF16, tag="yim")
                    for wb in range(64):
                        xt = xpool.tile([128, 512], BF16, tag="xt")
                        # [h=128p x2 chunks, (4w,64c)=256]
                        nc.sync.dma_start(
                            out=xt[:].rearrange("p (k w c) -> p k w c", k=2, w=4),
                            in_=xs[b, :, 4 * wb:4 * wb + 4, :]
                            .rearrange("(k p) w c -> p k w c", k=2))
                        pre = pspool.tile([128, 256], F32, tag="pre")
                        pim = pspool.tile([128, 256], F32, tag="pim")
                        ct, st = cons[f"ch{hf}"], cons[f"sh{hf}"]
                        nc.tensor.matmul(pre[:], ct[:, 0:128], xt[:, 0:256],
                                         start=True, stop=False)
                        nc.tensor.matmul(pre[:], ct[:, 128:256], xt[:, 256:512],
                                         start=False, stop=True)
                        nc.tensor.matmul(pim[:], st[:, 0:128], xt[:, 0:256],
                                         start=True, stop=False)
                        nc.tensor.matmul(pim[:], st[:, 128:256], xt[:, 256:512],
                                         start=False, stop=True)
                        if wb % 2 == 0:
                            nc.vector.tensor_copy(
                                yre[:, 256 * wb:256 * wb + 256], pre[:])
                            nc.scalar.copy(
                                yim[:, 256 * wb:256 * wb + 256], pim[:])
                        else:
                            nc.scalar.copy(
                                yre[:, 256 * wb:256 * wb + 256], pre[:])
                            nc.vector.tensor_copy(
                                yim[:, 256 * wb:256 * wb + 256], pim[:])

                    # ------- Q path + Z modes per c-group of 16 -------
                    trow = 0 if hf == 0 else 96
                    zsb = wpool.tile([32, 4096], BF16, tag="zsb")
                    for cg in range(4):
                        yg = wpool.tile([128, 4096], BF16, tag="yg")
                        # regroup: yg[t, ci*256 + w] = yim[t, w*64 + (16cg+ci)]
                        nc.vector.tensor_copy(
                            yg[:].rearrange("p (c w) -> p c w", c=16),
                            yim[:].rearrange("p (w c) -> p c w", c=64)
                            [:, 16 * cg:16 * cg + 16, :])
                        ygr = wpool.tile([128, 4096], BF16, tag="ygr")
                        nc.scalar.copy(
                            ygr[:].rearrange("p (c w) -> p c w", c=16),
                            yre[:].rearrange("p (w c) -> p c w", c=64)
                            [:, 16 * cg:16 * cg + 16, :])
                        ytr = wpool.tile([128, 2048], BF16, tag="ytr0")
                        ytr1 = wpool.tile([128, 2048], BF16, tag="ytr1")
                        # yre^T slices for Z: [w-chunk, ci*32+t(32)]
                        ztr = wpool.tile([128, 512], BF16, tag="ztr0")
                        ztr1 = wpool.tile([128, 512], BF16, tag="ztr1")
                        for ci in range(16):
                            for k in range(2):
                                ptr = psvpool.tile([128, 128], BF16, tag="ptr")
                                nc.tensor.transpose(
                                    ptr[:],
                                    yg[:, 256 * ci + 128 * k:256 * ci + 128 * k + 128],
                                    ident[:])
                                dst = ytr if k == 0 else ytr1
                                nc.vector.tensor_copy(
                                    dst[:, 128 * ci:128 * ci + 128], ptr[:])
                                ptz = psvpool.tile([128, 128], BF16, tag="ptr")
                                nc.tensor.transpose(
                                    ptz[:],
                                    ygr[:, 256 * ci + 128 * k:256 * ci + 128 * k + 128],
                                    ident[:])
                                dstz = ztr if k == 0 else ztr1
                                nc.scalar.copy(
                                    dstz[:, 32 * ci:32 * ci + 32],
                                    ptz[:, trow:trow + 32])
                        # Z matmuls for this c-group -> zsb cols [512cg:+512]
                        # rhs yim^T slices: strided view of ytr [w, (ci,t)]
                        yimt0 = ytr[:].rearrange("p (c t) -> p c t", c=16)[:, :, trow:trow + 32]
                        yimt1 = ytr1[:].rearrange("p (c t) -> p c t", c=16)[:, :, trow:trow + 32]
                        zr = pszpool.tile([32, 512], F32, tag="zz")
                        zi = pszpool.tile([32, 512], F32, tag="zz")
                        cwt, swt, swnt = zcons["cw"], zcons["sw"], zcons["swn"]
                        nc.tensor.matmul(zr[:], cwt[:, 0:32], ztr[:],
                                         start=True, stop=False)
                        nc.tensor.matmul(zr[:], cwt[:, 32:64], ztr1[:],
                                         start=False, stop=False)
                        nc.tensor.matmul(zr[:], swt[:, 0:32], yimt0,
                                         start=False, stop=False)
                        nc.tensor.matmul(zr[:], swt[:, 32:64], yimt1,
                                         start=False, stop=True)
                        nc.tensor.matmul(zi[:], cwt[:, 0:32], yimt0,
                                         start=True, stop=False)
                        nc.tensor.matmul(zi[:], cwt[:, 32:64], yimt1,
                                         start=False, stop=False)
                        nc.tensor.matmul(zi[:], swnt[:, 0:32], ztr[:],
                                         start=False, stop=False)
                        nc.tensor.matmul(zi[:], swnt[:, 32:64], ztr1[:],
                                         start=False, stop=True)
                        nc.vector.tensor_copy(
                            zsb[:, 512 * cg:512 * cg + 512], zr[:])
                        nc.scalar.copy(
                            zsb[:, 2048 + 512 * cg:2048 + 512 * cg + 512], zi[:])
                        # Q matmuls: add yim*Q into yre
                        for ci in range(16):
                            c = 16 * cg + ci
                            pv = pspool.tile([128, 256], F32, tag="pre")
                            nc.tensor.matmul(pv[:], ytr[:, 128 * ci:128 * ci + 128],
                                             qmt[:, 0:256], start=True, stop=False)
                            nc.tensor.matmul(pv[:], ytr1[:, 128 * ci:128 * ci + 128],
                                             qmt[:, 256:512], start=False, stop=True)
                            nc.vector.tensor_add(
                                yre[:].rearrange("p (w c) -> p c w", c=64)[:, c, :],
                                yre[:].rearrange("p (w c) -> p c w", c=64)[:, c, :],
                                pv[:])
                    nc.sync.dma_start(out=zout[b, hf], in_=zsb[:])
                    # out rows (bf16 cast, chunked staging)
                    for oc in range(4):
                        ob = opool.tile([128, 4096], BF16, tag="ob")
                        if oc % 2 == 0:
                            nc.vector.tensor_copy(ob[:], yre[:, 4096 * oc:4096 * oc + 4096])
                        else:
                            nc.scalar.copy(ob[:], yre[:, 4096 * oc:4096 * oc + 4096])
                        nc.sync.dma_start(
                            out=out[b, 128 * hf:128 * hf + 128,
                                    64 * oc:64 * oc + 64, :]
                            .rearrange("p w c -> p (w c)"),
                            in_=ob[:])
    nc.compile()
    return nc


def _get_runtime():
    if "rt" in _CACHE:
        return _CACHE["rt"]
    import jax
    from concourse.bass2jax import (
        _bass_exec_p, install_neuronx_cc_hook, partition_id_tensor)
    install_neuronx_cc_hook()
    nc = _build()
    partition_name = (nc.partition_id_tensor.name
                      if nc.partition_id_tensor is not None else None)
    in_names, out_names, out_avals = [], [], []
    for alloc in nc.m.functions[0].allocations:
        if not isinstance(alloc, mybir.MemoryLocationSet):
            continue
        assert alloc.memorylocations
        name = alloc.memorylocations[0].name
        if alloc.kind == "ExternalInput":
            if name != partition_name:
                in_names.append(name)
        elif alloc.kind == "ExternalOutput":
            shape = tuple(alloc.tensor_shape)
            dtype = mybir.dt.np(alloc.dtype)
            out_names.append(name)
            out_avals.append(jax.core.ShapedArray(shape, dtype))
    n_params = len(in_names)
    all_names = list(in_names) + list(out_names)
    if partition_name is not None:
        all_names.append(partition_name)

    def _body(*args):
        operands = list(args)
        if partition_name is not None:
            operands.append(partition_id_tensor())
        outs = _bass_exec_p.bind(
            *operands,
            out_avals=tuple(out_avals),
            in_names=tuple(all_names),
            out_names=tuple(out_names),
            lowering_input_output_aliases=(),
            sim_require_finite=True,
            sim_require_nnan=True,
            nc=nc,
        )
        return tuple(outs)

    devices = jax.devices()[:N_CORES]
    # per-device jit (no shard_map): each core's exec starts as soon as ITS
    # x shard lands, so early cores finish during the upload window
    fn = jax.jit(_body, keep_unused=True)
    cons = _constants()
    const_dev = [
        {k: jax.device_put(np.asarray(v), d) for k, v in cons.items()}
        for d in devices]
    zeros_dev = []
    for d in devices:
        zd = {}
        for name, aval in zip(out_names, out_avals):
            zd[name] = jax.device_put(
                np.zeros(tuple(aval.shape), aval.dtype), d)
        zeros_dev.append(zd)
    rt = (fn, in_names, out_names, devices, const_dev, zeros_dev)
    _CACHE["rt"] = rt
    return rt


def _get_wm(w1, w2):
    import zlib
    w1f = np.ascontiguousarray(w1, dtype=np.float32)
    w2f = np.ascontiguousarray(w2, dtype=np.float32)
    key = (zlib.adler32(memoryview(w1f).cast("B")),
           zlib.adler32(memoryview(w2f).cast("B")))
    hit = _CACHE.get("wm")
    if hit is not None and hit[0] == key:
        return hit[1]
    # einsum 'bctq,dctq->bdtq': weight axis0 = output channel d, axis1 = c
    w1c = (w1f[..., 0] + 1j * w1f[..., 1]).astype(np.complex64)
    w2c = (w2f[..., 0] + 1j * w2f[..., 1]).astype(np.complex64)
    Wm = np.stack([np.ascontiguousarray(w1c.transpose(2, 3, 1, 0)),
                   np.ascontiguousarray(w2c.transpose(2, 3, 1, 0))])  # blk,t,q,c,d
    _CACHE["wm"] = (key, Wm)
    return Wm


def _corr_rows(zraw, Wm):
    # zraw: [B, 2, 32, 4096] bf16; cols [0:2048]=Zre[q, c*32+t], [2048:]=Zim
    z = np.asarray(zraw).astype(np.float32)
    zre = z[..., :2048].reshape(B, 2, 32, 64, 32)  # b, blk, q, c, t
    zim = z[..., 2048:].reshape(B, 2, 32, 64, 32)
    Zc = zre + 1j * zim  # complex64
    A = np.ascontiguousarray(np.transpose(Zc, (1, 4, 2, 0, 3)))  # blk,t,q,b,c
    E = np.matmul(A, Wm)              # blk, t, q, b, d
    delta = (E - A).transpose(0, 1, 3, 2, 4)  # blk, t, b, q, d
    Xr = np.empty((2, 32, B, 64, 64), np.float32)
    Xr[..., 0::2, :] = delta.real
    Xr[..., 1::2, :] = delta.imag
    M = _CACHE.get("mext")
    if M is None:
        qv = np.arange(32)
        wv = np.arange(256)
        eps = (np.where(qv == 0, 1.0, 2.0) / 256.0).astype(np.float32)
        phi = 2 * np.pi * np.outer(wv, qv) / 256
        M = np.empty((256, 64), np.float32)
        M[:, 0::2] = np.cos(phi) * eps
        M[:, 1::2] = -np.sin(phi) * eps
        _CACHE["mext"] = M
    R = np.matmul(M, Xr)              # blk, t, b, w, d
    return R


def kernel(x, w1, w2):
    import jax
    import concurrent.futures as cf
    fn, in_names, out_names, devs, const_dev, zeros_dev = _get_runtime()
    oi = out_names.index("out")
    zi = out_names.index("zout")
    x = np.asarray(x)
    # per-device cast+put+dispatch pipeline: device i starts executing as
    # soon as its slice arrives, well before the full upload completes
    outs_per = []
    for i in range(N_CORES):
        xd = jax.device_put(x[BPC * i:BPC * i + BPC].astype(BF_NP), devs[i])
        args = [xd if n == "xs" else const_dev[i][n] for n in in_names]
        args += [zeros_dev[i][n] for n in out_names]
        outs_per.append(fn(*args))
    # weight prep overlaps the x upload + exec (link busy, CPU idle here)
    Wm = _get_wm(w1, w2)
    out = np.empty((B, H, W, C), np.float32)
    zraw = np.empty((B, 2, 32, 4096), BF_NP)

    def _fetch_out(i):
        out[BPC * i:BPC * i + BPC] = np.asarray(outs_per[i][oi])

    def _fetch_z(i):
        zraw[BPC * i:BPC * i + BPC] = np.asarray(outs_per[i][zi])

    # all fetches in parallel: overlaps per-RPC latency; corr einsum runs
    # while out shards stream in
    with cf.ThreadPoolExecutor(16) as ex:
        zf = [ex.submit(_fetch_z, i) for i in range(N_CORES)]
        of = [ex.submit(_fetch_out, i) for i in range(N_CORES)]
        for f in zf:
            f.result()
        R = _corr_rows(zraw, Wm)
        for f in of:
            f.result()
    out[:, 0:32] += R[0].transpose(1, 0, 2, 3)
    out[:, 224:256] += R[1].transpose(1, 0, 2, 3)
    return out


# revision 12
# speedup vs baseline: 1.1033x; 1.0090x over previous
import sys
sys.path.insert(0, "/opt/trn_rl_repo")
import numpy as np
import ml_dtypes
import concourse.bacc as bacc
import concourse.mybir as mybir
from concourse.tile import TileContext
from concourse.masks import make_identity

N_CORES = 8
B, H, W, C = 16, 256, 256, 64
BPC = B // N_CORES  # batches per core
F32 = mybir.dt.float32
BF16 = mybir.dt.bfloat16
BF_NP = ml_dtypes.bfloat16

_CACHE = {}


def _constants():
    t = np.arange(128)
    h = np.arange(256)
    out = {}
    for hf in range(2):
        ang = 2 * np.pi * (((t[None, :] + 128 * hf) * h[:, None]) % 256) / 256
        out[f"ch{hf}"] = np.cos(ang).astype(np.float32).astype(BF_NP)  # [h, t]
        out[f"sh{hf}"] = (-np.sin(ang)).astype(np.float32).astype(BF_NP)
    qm = np.fft.irfft(1j * np.fft.rfft(np.eye(256), axis=1), n=256, axis=1)
    out["qm"] = qm.astype(np.float32).astype(BF_NP)  # [w_in, w_out]
    wv = np.arange(256)
    qv = np.arange(32)
    phi = 2 * np.pi * np.outer(wv, qv) / 256  # [w, q]
    out["cw"] = np.cos(phi).astype(np.float32).astype(BF_NP)
    out["sw"] = np.sin(phi).astype(np.float32).astype(BF_NP)
    out["swn"] = (-np.sin(phi)).astype(np.float32).astype(BF_NP)
    return out


def _build():
    nc = bacc.Bacc()
    xs = nc.dram_tensor("xs", [BPC, H, W, C], BF16, kind="ExternalInput")
    ch0 = nc.dram_tensor("ch0", [256, 128], BF16, kind="ExternalInput")
    ch1 = nc.dram_tensor("ch1", [256, 128], BF16, kind="ExternalInput")
    sh0 = nc.dram_tensor("sh0", [256, 128], BF16, kind="ExternalInput")
    sh1 = nc.dram_tensor("sh1", [256, 128], BF16, kind="ExternalInput")
    qm = nc.dram_tensor("qm", [256, 256], BF16, kind="ExternalInput")
    cw = nc.dram_tensor("cw", [256, 32], BF16, kind="ExternalInput")
    sw = nc.dram_tensor("sw", [256, 32], BF16, kind="ExternalInput")
    swn = nc.dram_tensor("swn", [256, 32], BF16, kind="ExternalInput")
    out = nc.dram_tensor("out", [BPC, H, W, C], BF16, kind="ExternalOutput")
    # zout[b, hf, q, 0:2048] = Zre[q, c*32+t], [..., 2048:4096] = Zim
    zout = nc.dram_tensor("zout", [BPC, 2, 32, 4096], BF16, kind="ExternalOutput")
    chs = {0: ch0, 1: ch1}
    shs = {0: sh0, 1: sh1}

    with TileContext(nc) as tc:
        with tc.tile_pool(name="const", bufs=1) as cpool, \
             tc.tile_pool(name="big", bufs=1) as bigpool, \
             tc.tile_pool(name="xin", bufs=4) as xpool, \
             tc.tile_pool(name="work", bufs=1) as wpool, \
             tc.tile_pool(name="ob", bufs=2) as opool, \
             tc.tile_pool(name="ps", bufs=2, space="PSUM") as pspool, \
             tc.tile_pool(name="psv", bufs=2, space="PSUM") as psvpool, \
             tc.tile_pool(name="psz", bufs=2, space="PSUM") as pszpool:

            ident = cpool.tile([128, 128], BF16, tag="ident")
            make_identity(nc, ident[:])
            cons = {}
            for hf in range(2):
                for nm, src in (("ch", chs[hf]), ("sh", shs[hf])):
                    tl = cpool.tile([128, 256], BF16, tag=f"{nm}{hf}")
                    # [K=h(2x128 chunks), M=128t] stored as [128, 2*128]
                    nc.sync.dma_start(
                        out=tl[:].rearrange("p (k m) -> p k m", k=2),
                        in_=src[:].rearrange("(k p) m -> p k m", k=2))
                    cons[f"{nm}{hf}"] = tl
            qmt = cpool.tile([128, 512], BF16, tag="qm")
            nc.sync.dma_start(
                out=qmt[:].rearrange("p (k m) -> p k m", k=2),
                in_=qm[:].rearrange("(k p) m -> p k m", k=2))
            # rfft-mode tables [K=w (2x128 chunks), M=32q] as [128, 2*32]
            zcons = {}
            for nm, src in (("cw", cw), ("sw", sw), ("swn", swn)):
                tl = cpool.tile([128, 64], BF16, tag=nm)
                nc.sync.dma_start(
                    out=tl[:].rearrange("p (k q) -> p k q", k=2),
                    in_=src[:].rearrange("(k p) q -> p k q", k=2))
                zcons[nm] = tl

            for b in range(BPC):
                for hf in range(2):
                    # ---------------- phase B: contract h ----------------
                    yre = bigpool.tile([128, 16384], F32, tag="yre")
                    yim = bigpool.tile([128, 16384], BF16, tag="yim")
                    for wb in range(64):
                        xt = xpool.tile([128, 512], BF16, tag="xt")
                        # [h=128p x2 chunks, (4w,64c)=256]
                        nc.sync.dma_start(
                            out=xt[:].rearrange("p (k w c) -> p k w c", k=2, w=4),
                            in_=xs[b, :, 4 * wb:4 * wb + 4, :]
                            .rearrange("(k p) w c -> p k w c", k=2))
                        pre = pspool.tile([128, 256], F32, tag="pre")
                        pim = pspool.tile([128, 256], F32, tag="pim")
                        ct, st = cons[f"ch{hf}"], cons[f"sh{hf}"]
                        nc.tensor.matmul(pre[:], ct[:, 0:128], xt[:, 0:256],
                                         start=True, stop=False)
                        nc.tensor.matmul(pre[:], ct[:, 128:256], xt[:, 256:512],
                                         start=False, stop=True)
                        nc.tensor.matmul(pim[:], st[:, 0:128], xt[:, 0:256],
                                         start=True, stop=False)
                        nc.tensor.matmul(pim[:], st[:, 128:256], xt[:, 256:512],
                                         start=False, stop=True)
                        if wb % 2 == 0:
                            nc.vector.tensor_copy(
                                yre[:, 256 * wb:256 * wb + 256], pre[:])
                            nc.scalar.copy(
                                yim[:, 256 * wb:256 * wb + 256], pim[:])
                        else:
                            nc.scalar.copy(
                                yre[:, 256 * wb:256 * wb + 256], pre[:])
                            nc.vector.tensor_copy(
                                yim[:, 256 * wb:256 * wb + 256], pim[:])

                    # ------- Q path + Z modes per c-group of 16 -------
                    trow = 0 if hf == 0 else 96
                    zsb = wpool.tile([32, 4096], BF16, tag="zsb")
                    for cg in range(4):
                        yg = wpool.tile([128, 4096], BF16, tag="yg")
                        # regroup: yg[t, ci*256 + w] = yim[t, w*64 + (16cg+ci)]
                        nc.vector.tensor_copy(
                            yg[:].rearrange("p (c w) -> p c w", c=16),
                            yim[:].rearrange("p (w c) -> p c w", c=64)
                            [:, 16 * cg:16 * cg + 16, :])
                        ygr = wpool.tile([128, 4096], BF16, tag="ygr")
                        nc.scalar.copy(
                            ygr[:].rearrange("p (c w) -> p c w", c=16),
                            yre[:].rearrange("p (w c) -> p c w", c=64)
                            [:, 16 * cg:16 * cg + 16, :])
                        ytr = wpool.tile([128, 2048], BF16, tag="ytr0")
                        ytr1 = wpool.tile([128, 2048], BF16, tag="ytr1")
                        # yre^T slices for Z: [w-chunk, ci*32+t(32)]
                        ztr = wpool.tile([128, 512], BF16, tag="ztr0")
                        ztr1 = wpool.tile([128, 512], BF16, tag="ztr1")
                        for ci in range(16):
                            for k in range(2):
                                ptr = psvpool.tile([128, 128], BF16, tag="ptr")
                                nc.tensor.transpose(
                                    ptr[:],
                                    yg[:, 256 * ci + 128 * k:256 * ci + 128 * k + 128],
                                    ident[:])
                                dst = ytr if k == 0 else ytr1
                                nc.vector.tensor_copy(
                                    dst[:, 128 * ci:128 * ci + 128], ptr[:])
                                ptz = psvpool.tile([128, 128], BF16, tag="ptr")
                                nc.tensor.transpose(
                                    ptz[:],
                                    ygr[:, 256 * ci + 128 * k:256 * ci + 128 * k + 128],
                                    ident[:])
                                dstz = ztr if k == 0 else ztr1
                                nc.scalar.copy(
                                    dstz[:, 32 * ci:32 * ci + 32],
                                    ptz[:, trow:trow + 32])
                        # Z matmuls for this c-group -> zsb cols [512cg:+512]
                        # rhs yim^T slices: strided view of ytr [w, (ci,t)]
                        yimt0 = ytr[:].rearrange("p (c t) -> p c t", c=16)[:, :, trow:trow + 32]
                        yimt1 = ytr1[:].rearrange("p (c t) -> p c t", c=16)[:, :, trow:trow + 32]
                        zr = pszpool.tile([32, 512], F32, tag="zz")
                        zi = pszpool.tile([32, 512], F32, tag="zz")
                        cwt, swt, swnt = zcons["cw"], zcons["sw"], zcons["swn"]
                        nc.tensor.matmul(zr[:], cwt[:, 0:32], ztr[:],
                                         start=True, stop=False)
                        nc.tensor.matmul(zr[:], cwt[:, 32:64], ztr1[:],
                                         start=False, stop=False)
                        nc.tensor.matmul(zr[:], swt[:, 0:32], yimt0,
                                         start=False, stop=False)
                        nc.tensor.matmul(zr[:], swt[:, 32:64], yimt1,
                                         start=False, stop=True)
                        nc.tensor.matmul(zi[:], cwt[:, 0:32], yimt0,
                                         start=True, stop=False)
                        nc.tensor.matmul(zi[:], cwt[:, 32:64], yimt1,
                                         start=False, stop=False)
                        nc.tensor.matmul(zi[:], swnt[:, 0:32], ztr[:],
                                         start=False, stop=False)
                        nc.tensor.matmul(zi[:], swnt[:, 32:64], ztr1[:],
                                         start=False, stop=True)
                        nc.vector.tensor_copy(
                            zsb[:, 512 * cg:512 * cg + 512], zr[:])
                        nc.scalar.copy(
                            zsb[:, 2048 + 512 * cg:2048 + 512 * cg + 512], zi[:])
                        # Q matmuls: add yim*Q into yre
                        for ci in range(16):
                            c = 16 * cg + ci
                            pv = pspool.tile([128, 256], F32, tag="pre")
                            nc.tensor.matmul(pv[:], ytr[:, 128 * ci:128 * ci + 128],
                                             qmt[:, 0:256], start=True, stop=False)
                            nc.tensor.matmul(pv[:], ytr1[:, 128 * ci:128 * ci + 128],
                                             qmt[:, 256:512], start=False, stop=True)
                            nc.vector.tensor_add(
                                yre[:].rearrange("p (w c) -> p c w", c=64)[:, c, :],
                                yre[:].rearrange("p (w c) -> p c w", c=64)[:, c, :],
                                pv[:])
                    nc.sync.dma_start(out=zout[b, hf], in_=zsb[:])
                    # out rows (bf16 cast, chunked staging)
                    for oc in range(4):
                        ob = opool.tile([128, 4096], BF16, tag="ob")
                        if oc % 2 == 0:
                            nc.vector.tensor_copy(ob[:], yre[:, 4096 * oc:4096 * oc + 4096])
                        else:
                            nc.scalar.copy(ob[:], yre[:, 4096 * oc:4096 * oc + 4096])
                        nc.sync.dma_start(
                            out=out[b, 128 * hf:128 * hf + 128,
                                    64 * oc:64 * oc + 64, :]
                            .rearrange("p w c -> p (w c)"),
                            in_=ob[:])
    nc.compile()
    return nc


def _get_runtime():
    if "rt" in _CACHE:
        return _CACHE["rt"]
    import jax
    from concourse.bass2jax import (
        _bass_exec_p, install_neuronx_cc_hook, partition_id_tensor)
    install_neuronx_cc_hook()
    nc = _build()
    partition_name = (nc.partition_id_tensor.name
                      if nc.partition_id_tensor is not None else None)
    in_names, out_names, out_avals = [], [], []
    for alloc in nc.m.functions[0].allocations:
        if not isinstance(alloc, mybir.MemoryLocationSet):
            continue
        assert alloc.memorylocations
        name = alloc.memorylocations[0].name
        if alloc.kind == "ExternalInput":
            if name != partition_name:
                in_names.append(name)
        elif alloc.kind == "ExternalOutput":
            shape = tuple(alloc.tensor_shape)
            dtype = mybir.dt.np(alloc.dtype)
            out_names.append(name)
            out_avals.append(jax.core.ShapedArray(shape, dtype))
    n_params = len(in_names)
    all_names = list(in_names) + list(out_names)
    if partition_name is not None:
        all_names.append(partition_name)

    def _body(*args):
        operands = list(args)
        if partition_name is not None:
            operands.append(partition_id_tensor())
        outs = _bass_exec_p.bind(
            *operands,
            out_avals=tuple(out_avals),
            in_names=tuple(all_names),
            out_names=tuple(out_names),
            lowering_input_output_aliases=(),
            sim_require_finite=True,
            sim_require_nnan=True,
            nc=nc,
        )
        return tuple(outs)

    devices = jax.devices()[:N_CORES]
    # per-device jit (no shard_map): each core's exec starts as soon as ITS
    # x shard lands, so early cores finish during the upload window
    fn = jax.jit(_body, keep_unused=True)
    cons = _constants()
    const_dev = [
        {k: jax.device_put(np.asarray(v), d) for k, v in cons.items()}
        for d in devices]
    zeros_dev = []
    for d in devices:
        zd = {}
        for name, aval in zip(out_names, out_avals):
            zd[name] = jax.device_put(
                np.zeros(tuple(aval.shape), aval.dtype), d)
        zeros_dev.append(zd)
    rt = (fn, in_names, out_names, devices, const_dev, zeros_dev)
    _CACHE["rt"] = rt
    return rt


def _get_wm(w1, w2):
    import zlib
    w1f = np.ascontiguousarray(w1, dtype=np.float32)
    w2f = np.ascontiguousarray(w2, dtype=np.float32)
    key = (zlib.adler32(memoryview(w1f).cast("B")),
           zlib.adler32(memoryview(w2f).cast("B")))
    hit = _CACHE.get("wm")
    if hit is not None and hit[0] == key:
        return hit[1]
    # einsum 'bctq,dctq->bdtq': weight axis0 = output channel d, axis1 = c
    w1c = (w1f[..., 0] + 1j * w1f[..., 1]).astype(np.complex64)
    w2c = (w2f[..., 0] + 1j * w2f[..., 1]).astype(np.complex64)
    Wm = np.stack([np.ascontiguousarray(w1c.transpose(2, 3, 1, 0)),
                   np.ascontiguousarray(w2c.transpose(2, 3, 1, 0))])  # blk,t,q,c,d
    _CACHE["wm"] = (key, Wm)
    return Wm


def _corr_rows(zraw, Wm):
    # zraw: [B, 2, 32, 4096] bf16; cols [0:2048]=Zre[q, c*32+t], [2048:]=Zim
    z = np.asarray(zraw).astype(np.float32)
    zre = z[..., :2048].reshape(B, 2, 32, 64, 32)  # b, blk, q, c, t
    zim = z[..., 2048:].reshape(B, 2, 32, 64, 32)
    Zc = zre + 1j * zim  # complex64
    A = np.ascontiguousarray(np.transpose(Zc, (1, 4, 2, 0, 3)))  # blk,t,q,b,c
    E = np.matmul(A, Wm)              # blk, t, q, b, d
    delta = (E - A).transpose(0, 1, 3, 2, 4)  # blk, t, b, q, d
    Xr = np.empty((2, 32, B, 64, 64), np.float32)
    Xr[..., 0::2, :] = delta.real
    Xr[..., 1::2, :] = delta.imag
    M = _CACHE.get("mext")
    if M is None:
        qv = np.arange(32)
        wv = np.arange(256)
        eps = (np.where(qv == 0, 1.0, 2.0) / 256.0).astype(np.float32)
        phi = 2 * np.pi * np.outer(wv, qv) / 256
        M = np.empty((256, 64), np.float32)
        M[:, 0::2] = np.cos(phi) * eps
        M[:, 1::2] = -np.sin(phi) * eps
        _CACHE["mext"] = M
    R = np.matmul(M, Xr)              # blk, t, b, w, d
    return R


def kernel(x, w1, w2):
    import jax
    import concurrent.futures as cf
    fn, in_names, out_names, devs, const_dev, zeros_dev = _get_runtime()
    oi = out_names.index("out")
    zi = out_names.index("zout")
    x = np.asarray(x)
    # per-device cast+put+dispatch pipeline: device i starts executing as
    # soon as its slice arrives, well before the full upload completes
    outs_per = []
    for i in range(N_CORES):
        xd = jax.device_put(x[BPC * i:BPC * i + BPC].astype(BF_NP), devs[i])
        args = [xd if n == "xs" else const_dev[i][n] for n in in_names]
        args += [zeros_dev[i][n] for n in out_names]
        outs_per.append(fn(*args))
    # weight prep + result-buffer page-faulting overlap the x upload + exec
    # (link busy, CPU idle here; faulting later would stall the fetch pump)
    Wm = _get_wm(w1, w2)
    out = np.empty((B, H, W, C), np.float32)
    out.fill(0.0)
    zraw = np.empty((B, 2, 32, 4096), BF_NP)

    def _fetch_out(i):
        out[BPC * i:BPC * i + BPC] = np.asarray(outs_per[i][oi])

    def _fetch_z(i):
        zraw[BPC * i:BPC * i + BPC] = np.asarray(outs_per[i][zi])

    # all fetches in parallel: overlaps per-RPC latency; corr einsum runs
    # while out shards stream in
    with cf.ThreadPoolExecutor(16) as ex:
        zf = [ex.submit(_fetch_z, i) for i in range(N_CORES)]
        of = [ex.submit(_fetch_out, i) for i in range(N_CORES)]
        for f in zf:
            f.result()
        R = _corr_rows(zraw, Wm)
        for f in of:
            f.result()
    out[:, 0:32] += R[0].transpose(1, 0, 2, 3)
    out[:, 224:256] += R[1].transpose(1, 0, 2, 3)
    return out
